# revision 7
# baseline (speedup 1.0000x reference)
"""3-layer GAT on 8 Trainium2 NeuronCores.

Strategy: destination-node sharding. Edges (+self-loops) are sorted by dst and
partitioned into 8 shards of 2500 dst nodes; each shard is split into windows
of 128 dst nodes. Per window, h-rows of edge sources are fetched with
dma_gather (edge-major: edge -> partition), attention logits are computed per
edge, and the segment-softmax + weighted aggregation are realized as one-hot
matmuls accumulating in PSUM (U = sum_e ex*h[src], s = sum_e ex, out = U/s).
Per-node tables [h | al_src | al_dst] are produced by a dense matmul
act @ [W | W@as | W@ad] on every core; activations are exchanged between
layers with an AllGather of transposed shards.
"""
import sys

for _p in ("/opt/trn_rl_repo",):
    if _p not in sys.path:
        sys.path.insert(0, _p)

import numpy as np

import concourse.bacc as bacc
import concourse.bass as bass
import concourse.mybir as mybir
import concourse.tile as tile
from concourse import bass_utils
from concourse.library_config import mlp

F32 = mybir.dt.float32
I16 = mybir.dt.int16
ALU = mybir.AluOpType
ACTF = mybir.ActivationFunctionType

NEG_SLOPE = 0.2
EPS = 1e-16


class Cfg:
    def __init__(self, N=20000, IN=128, HID=64, HEADS=4, OUT=64, NC=8):
        assert N % NC == 0
        self.N, self.IN, self.HID, self.HEADS, self.OUT, self.NC = (
            N, IN, HID, HEADS, OUT, NC)
        self.NLOC = N // NC
        self.NWIN = -(-self.NLOC // 128)
        self.NPAD = self.NWIN * 128
        self.NTOT = NC * self.NPAD
        self.F1 = HEADS * HID                      # 256
        # per-layer: (F_in, F_out, heads, row_width, relu)
        self.layers = [
            (IN, self.F1, HEADS, self._rw(self.F1, HEADS), True),
            (self.F1, self.F1, HEADS, self._rw(self.F1, HEADS), True),
            (self.F1, OUT, 1, self._rw(OUT, 1), False),
        ]

    @staticmethod
    def _rw(fout, heads):
        # row = [h (fout) | als (heads) | ald (heads) | pad]; f32 rows must be
        # a multiple of 64 elems (256B) for dma_gather, and we need a 64-elem
        # aligned slice containing [als|ald] for the by-dst side gather.
        need = fout + 2 * heads
        rw = ((need + 63) // 64) * 64
        if rw - fout < 64:
            rw = fout + 64
        return rw


def _wrap16(idx_flat):
    """[M] -> [128, M//16] int16 index layout for gpsimd dma_gather."""
    a = np.asarray(idx_flat, np.int16).reshape(-1, 16).T
    return np.ascontiguousarray(np.tile(a, (8, 1)))


def host_prep(cfg, x, edge_index, weights):
    """weights: dict W0,as0,ad0,b0,... Returns (shared_inputs, per_core_inputs, Tw)."""
    N, NC, NLOC, NWIN = cfg.N, cfg.NC, cfg.NLOC, cfg.NWIN
    src = np.concatenate([edge_index[0], np.arange(N, dtype=np.int64)])
    dst = np.concatenate([edge_index[1], np.arange(N, dtype=np.int64)])
    order = np.argsort(dst, kind="stable")
    src, dst = src[order], dst[order]

    # bucket edges by (core, window)
    core_of = dst // NLOC
    wloc = (dst % NLOC) // 128
    dloc = (dst % NLOC) % 128

    # counts[c, w]
    counts = np.zeros((NC, NWIN), np.int64)
    np.add.at(counts, (core_of, wloc), 1)
    Tw = int(-(-counts.max() // 128))

    # H-table row id for any real node id
    def row_of(v):
        return (v // NLOC) * cfg.NPAD + (v % NLOC)

    per_core = []
    ES = NWIN * Tw * 128  # padded edge slots per core
    for c in range(NC):
        gsrc = np.zeros(ES, np.int64)
        gdst = np.zeros(ES, np.int64)
        dl = np.full(ES, 200.0, np.float32)
        m = core_of == c
        sc, dc, wc, dlc = src[m], dst[m], wloc[m], dloc[m]
        for w in range(NWIN):
            wm = wc == w
            n = int(wm.sum())
            base = w * Tw * 128
            gsrc[base:base + n] = row_of(sc[wm])
            gdst[base:base + n] = row_of(dc[wm])
            dl[base:base + n] = dlc[wm].astype(np.float32)
        # edge-major [128, NWIN*Tw] view of dst_local: slot i -> [i%128, i//128]
        dl_em = np.ascontiguousarray(dl.reshape(NWIN * Tw, 128).T)
        per_core.append({
            "gidx_h": _wrap16(gsrc),
            "gidx_d": _wrap16(gdst),
            "dstloc": dl_em,
        })

    # shared inputs
    xT = np.zeros((cfg.IN, cfg.NTOT), np.float32)
    xv = np.asarray(x, np.float32)
    for c in range(NC):
        xT[:, c * cfg.NPAD:c * cfg.NPAD + NLOC] = xv[c * NLOC:(c + 1) * NLOC].T

    def wcat(W, a_s, a_d, heads, hid):
        W = np.asarray(W, np.float32)
        a_s = np.asarray(a_s, np.float32).reshape(heads, hid)
        a_d = np.asarray(a_d, np.float32).reshape(heads, hid)
        was = np.zeros((W.shape[0], heads), np.float32)
        wad = np.zeros((W.shape[0], heads), np.float32)
        for h in range(heads):
            was[:, h] = W[:, h * hid:(h + 1) * hid] @ a_s[h]
            wad[:, h] = W[:, h * hid:(h + 1) * hid] @ a_d[h]
        cat = np.concatenate([W, was, wad], axis=1)  # [F_in, F_out+2H]
        KB = W.shape[0] // 128
        # pack [F_in, NW2] -> [128, KB*NW2]
        return np.concatenate([cat[kb * 128:(kb + 1) * 128] for kb in range(KB)],
                              axis=1).astype(np.float32)

    H, HID_, OUT_ = cfg.HEADS, cfg.HID, cfg.OUT
    shared = {
        "xT": xT,
        "wcat0": wcat(weights["W0"], weights["as0"], weights["ad0"], H, HID_),
        "wcat1": wcat(weights["W1"], weights["as1"], weights["ad1"], H, HID_),
        "wcat2": wcat(weights["W2"], weights["as2"], weights["ad2"], 1, OUT_),
        "iota": np.tile(np.arange(128, dtype=np.float32)[None, :], (128, 1)),
        "ident": np.eye(128, dtype=np.float32),
        "b0r": np.tile(np.asarray(weights["b0"], np.float32)[None, :], (128, 1)),
        "b1r": np.tile(np.asarray(weights["b1"], np.float32)[None, :], (128, 1)),
        "b2r": np.tile(np.asarray(weights["b2"], np.float32)[None, :], (128, 1)),
    }
    return shared, per_core, Tw


def build_module(cfg, Tw):
    nc = bacc.Bacc("TRN2", target_bir_lowering=False, debug=False,
                   num_devices=cfg.NC)
    NWIN, NPAD, NTOT, NC = cfg.NWIN, cfg.NPAD, cfg.NTOT, cfg.NC
    ES = NWIN * Tw * 128

    def din(name, shape, dtype=F32):
        return nc.dram_tensor(name, list(shape), dtype, kind="ExternalInput")

    xT = din("xT", (cfg.IN, NTOT))
    wc = [din("wcat0", (128, cfg.F1 + 2 * cfg.HEADS)),
          din("wcat1", (128, 2 * (cfg.F1 + 2 * cfg.HEADS))),
          din("wcat2", (128, 2 * (cfg.OUT + 2)))]
    iota = din("iota", (128, 128))
    ident = din("ident", (128, 128))
    brep = [din("b0r", (128, cfg.F1)), din("b1r", (128, cfg.F1)),
            din("b2r", (128, cfg.OUT))]
    gidx_h = din("gidx_h", (128, ES // 16), I16)
    gidx_d = din("gidx_d", (128, ES // 16), I16)
    dstloc = din("dstloc", (128, NWIN * Tw))

    out_d = nc.dram_tensor("out", [NPAD, cfg.OUT], F32, kind="ExternalOutput")

    with tile.TileContext(nc) as tc:
        with (
            tc.tile_pool(name="const", bufs=1) as cp,
            tc.tile_pool(name="work", bufs=2) as wp,
            tc.tile_pool(name="lt", bufs=3) as ltp,
            tc.tile_pool(name="stage", bufs=3) as sp,
            tc.tile_pool(name="psum", bufs=2, space="PSUM") as pp,
            tc.tile_pool(name="psd", bufs=2, space="PSUM") as pdp,
            tc.tile_pool(name="pst", bufs=2, space="PSUM") as ptp,
            tc.tile_pool(name="dram", bufs=1, space="DRAM") as dp,
        ):
            # internal DRAM scratch (pool tiles => dependency-tracked)
            ht = [dp.tile([NTOT, cfg.layers[l][3]], F32, name=f"ht{l}",
                          tag=f"ht{l}") for l in range(3)]
            ag_in = [dp.tile([2, 128, NPAD], F32, name=f"agin{l}",
                             tag=f"agin{l}") for l in range(2)]
            ag_out = [dp.tile([NC, 2, 128, NPAD], F32, name=f"agout{l}",
                              tag=f"agout{l}") for l in range(2)]
            nc.gpsimd.load_library(mlp)

            # ---- load constants to SBUF ----
            def load_const(dram, shape, dtype=F32):
                t = cp.tile(list(shape), dtype, tag=dram.name)
                nc.sync.dma_start(t[:], dram.ap())
                return t

            wcs = [load_const(wc[0], (128, cfg.F1 + 2 * cfg.HEADS)),
                   load_const(wc[1], (128, 2 * (cfg.F1 + 2 * cfg.HEADS))),
                   load_const(wc[2], (128, 2 * (cfg.OUT + 2)))]
            iot = load_const(iota, (128, 128))
            idn = load_const(ident, (128, 128))
            brs = [load_const(brep[0], (128, cfg.F1)),
                   load_const(brep[1], (128, cfg.F1)),
                   load_const(brep[2], (128, cfg.OUT))]
            gih = load_const(gidx_h, (128, ES // 16), I16)
            gid = load_const(gidx_d, (128, ES // 16), I16)
            dsl = load_const(dstloc, (128, NWIN * Tw))

            for l, (fin, fout, H, RW, relu) in enumerate(cfg.layers):
                KB = fin // 128
                NW2 = fout + 2 * H

                # ---------- dense phase: tables [h | als | ald] ----------
                for nt in range(NTOT // 128):
                    psd = pdp.tile([128, NW2], F32, tag="psd")
                    for kb in range(KB):
                        lt = ltp.tile([128, 128], F32, tag="lt")
                        if l == 0:
                            src_ap = xT[:, nt * 128:(nt + 1) * 128]
                        else:
                            c, s = nt // NWIN, nt % NWIN
                            src_ap = ag_out[l - 1][c, kb, :,
                                                   s * 128:(s + 1) * 128]
                        nc.sync.dma_start(lt[:], src_ap)
                        nc.tensor.matmul(
                            psd[:], lt[:],
                            wcs[l][:, kb * NW2:(kb + 1) * NW2],
                            start=(kb == 0), stop=(kb == KB - 1))
                    stg = sp.tile([128, NW2], F32, tag="hrow")
                    nc.scalar.activation(stg[:], psd[:], ACTF.Copy)
                    nc.sync.dma_start(
                        ht[l][nt * 128:(nt + 1) * 128, 0:NW2], stg[:])

                # ---------- edge phase ----------
                C0 = fout            # als col offset in row
                # 64-aligned slice start containing [als|ald]
                s64 = (C0 // 64) * 64
                rel = C0 - s64       # als offset within the 64-slice
                assert C0 + 2 * H <= s64 + 64
                # chunk gathers: SWDGE desc ring holds ~256 descs/engine;
                # keep num_idxs/16+1 well under that.
                GCH = 8  # tiles (=1024 idxs, 65 descs) per dma_gather
                for w in range(NWIN):
                    g1 = wp.tile([128, Tw, RW], F32, tag="g1")
                    g2 = wp.tile([128, Tw, 64], F32, tag="g2")
                    for t0 in range(0, Tw, GCH):
                        ch = min(GCH, Tw - t0)
                        isl = slice((w * Tw + t0) * 8, (w * Tw + t0 + ch) * 8)
                        nc.gpsimd.dma_gather(
                            g1[:, t0:t0 + ch, :], ht[l][:], gih[:, isl],
                            ch * 128, ch * 128, RW)
                        nc.gpsimd.dma_gather(
                            g2[:, t0:t0 + ch, :], ht[l][:, s64:s64 + 64],
                            gid[:, isl], ch * 128, ch * 128, 64, elem_step=RW)

                    als = g1[:, :, C0:C0 + H]
                    ald = g2[:, :, rel + H:rel + 2 * H]
                    zb = wp.tile([128, Tw, H], F32, tag="zb")
                    nc.vector.tensor_tensor(zb[:], als, ald, ALU.add)
                    zb2 = wp.tile([128, Tw, H], F32, tag="zb2")
                    nc.vector.tensor_scalar_mul(zb2[:], zb[:], NEG_SLOPE)
                    nc.vector.tensor_tensor(zb2[:], zb2[:], zb[:], ALU.max)
                    # ex -> overwrite als columns of g1
                    nc.scalar.activation(als, zb2[:], ACTF.Exp)
                    # fold ex into h columns (in place)
                    g1_4d = g1[:, :, 0:fout].rearrange(
                        "p t (h c) -> p t h c", h=H)
                    exb = als.unsqueeze(3).broadcast_to(
                        (128, Tw, H, fout // H))
                    nc.vector.tensor_tensor(g1_4d, g1_4d, exb, ALU.mult)

                    ps = pp.tile([128, fout + H], F32, tag="ps")
                    for t in range(Tw):
                        s_t = wp.tile([128, 128], F32, tag="s_t")
                        nc.vector.tensor_scalar(
                            s_t[:], iot[:],
                            dsl[:, w * Tw + t:w * Tw + t + 1], None,
                            ALU.is_equal)
                        nc.tensor.matmul(ps[:], s_t[:],
                                         g1[:, t, 0:fout + H],
                                         start=(t == 0), stop=(t == Tw - 1))

                    seps = wp.tile([128, H], F32, tag="seps")
                    nc.vector.tensor_scalar_add(seps[:], ps[:, fout:fout + H],
                                                EPS)
                    rs = wp.tile([128, H], F32, tag="rs")
                    nc.vector.reciprocal(rs[:], seps[:])

                    ow = wp.tile([128, fout], F32, tag="ow")
                    if H > 1:
                        u4 = ps[:, 0:fout].rearrange("p (h c) -> p h c", h=H)
                        o4 = ow[:, :].rearrange("p (h c) -> p h c", h=H)
                        rsb = rs[:, :].unsqueeze(2).broadcast_to(
                            (128, H, fout // H))
                        nc.vector.tensor_tensor(o4, u4, rsb, ALU.mult)
                    else:
                        nc.vector.tensor_scalar(
                            ow[:], ps[:, 0:fout], rs[:, 0:1], None, ALU.mult)
                    nc.vector.tensor_tensor(ow[:], ow[:], brs[l][:], ALU.add)
                    if relu:
                        nc.vector.tensor_scalar_max(ow[:], ow[:], 0.0)

                    if l < 2:
                        for half in range(2):
                            pst = ptp.tile([128, 128], F32, tag="pst")
                            nc.tensor.transpose(
                                pst[:], ow[:, half * 128:(half + 1) * 128],
                                idn[:])
                            ts = sp.tile([128, 128], F32, tag="ts")
                            nc.scalar.activation(ts[:], pst[:], ACTF.Copy)
                            nc.sync.dma_start(
                                ag_in[l][half, :, w * 128:(w + 1) * 128],
                                ts[:])
                    else:
                        nc.sync.dma_start(
                            out_d[w * 128:(w + 1) * 128, :], ow[:])

                if l < 2:
                    nc.gpsimd.collective_compute(
                        "AllGather", ALU.bypass,
                        replica_groups=[list(range(NC))],
                        ins=[ag_in[l].opt()],
                        outs=[ag_out[l].opt()])

    nc.compile()
    return nc


# ---------------------------------------------------------------------------

_HARD_CFG = Cfg()


def kernel_run(inputs, trace=False, trace_kwargs=None):
    cfg = _HARD_CFG
    x = np.asarray(inputs["x"])
    ei = np.asarray(inputs["edge_index"])
    shared, per_core, Tw = host_prep(cfg, x, ei, inputs)
    nc = build_module(cfg, Tw)
    in_maps = []
    for c in range(cfg.NC):
        m = dict(shared)
        m.update(per_core[c])
        in_maps.append(m)
    res = bass_utils.run_bass_kernel_spmd(
        nc, in_maps, core_ids=list(range(cfg.NC)), trace=trace,
        **(trace_kwargs or {}))
    out = np.concatenate(
        [res.results[c]["out"][:cfg.NLOC] for c in range(cfg.NC)], axis=0)
    return out.astype(np.float32), res


def kernel(**inputs):
    return kernel_run(inputs)[0]


# revision 10
# speedup vs baseline: 5.5885x; 5.5885x over previous
"""3-layer GAT on 8 Trainium2 NeuronCores.

Strategy: destination-node sharding. Edges (+self-loops) are sorted by dst and
partitioned into 8 shards of 2500 dst nodes; each shard is split into windows
of 128 dst nodes. Per window, bf16 rows [h | al_src | al_dst] of edge sources
are fetched with dma_gather (edge-major: edge -> partition), a second small
gather fetches al_dst by destination, attention weights ex=exp(lrelu(.)) are
folded into the gathered features, and segment-softmax + aggregation run as
one-hot matmuls (host-precomputed fp8 masks) accumulating in PSUM:
U = sum_e ex*h[src], s = sum_e ex, out = U * recip(s).
Per-node tables are produced by a dense matmul act @ [W | W@as | W@ad] on
every core; activations cross layers via an AllGather of transposed shards.
"""
import sys

for _p in ("/opt/trn_rl_repo",):
    if _p not in sys.path:
        sys.path.insert(0, _p)

import ml_dtypes
import numpy as np

import concourse.bacc as bacc
import concourse.bass as bass
import concourse.mybir as mybir
import concourse.tile as tile
from concourse import bass_utils
from concourse.library_config import mlp

F32 = mybir.dt.float32
BF16 = mybir.dt.bfloat16
FP8 = mybir.dt.float8e4
I16 = mybir.dt.int16
ALU = mybir.AluOpType
ACTF = mybir.ActivationFunctionType
BF = ml_dtypes.bfloat16
F8 = ml_dtypes.float8_e4m3

NEG_SLOPE = 0.2
EPS = 1e-16
GCH = 16            # gather chunk (tiles of 128 idxs) per dma_gather


class Cfg:
    def __init__(self, N=20000, IN=128, HID=64, HEADS=4, OUT=64, NC=8):
        assert N % NC == 0
        self.N, self.IN, self.HID, self.HEADS, self.OUT, self.NC = (
            N, IN, HID, HEADS, OUT, NC)
        self.NLOC = N // NC
        self.NWIN = -(-self.NLOC // 128)
        self.NPAD = self.NWIN * 128
        self.NTOT = NC * self.NPAD
        self.F1 = HEADS * HID                      # 256
        # per-layer: (F_in, F_out, heads, row_width(bf16), relu)
        self.layers = [
            (IN, self.F1, HEADS, self._rw(self.F1, HEADS), True),
            (self.F1, self.F1, HEADS, self._rw(self.F1, HEADS), True),
            (self.F1, OUT, 1, self._rw(OUT, 1), False),
        ]

    @staticmethod
    def _rw(fout, heads):
        # bf16 rows must be a multiple of 128 elems (256B) for dma_gather
        need = fout + 2 * heads
        return ((need + 127) // 128) * 128


def _wrap16(idx_flat):
    """[M] -> [128, M//16] int16 index layout for gpsimd dma_gather."""
    a = np.asarray(idx_flat, np.int16).reshape(-1, 16).T
    return np.ascontiguousarray(np.tile(a, (8, 1)))


def host_prep(cfg, x, edge_index, weights):
    N, NC, NLOC, NWIN = cfg.N, cfg.NC, cfg.NLOC, cfg.NWIN
    src = np.concatenate([edge_index[0], np.arange(N, dtype=np.int64)])
    dst = np.concatenate([edge_index[1], np.arange(N, dtype=np.int64)])
    order = np.argsort(dst, kind="stable")
    src, dst = src[order], dst[order]

    core_of = dst // NLOC
    wloc = (dst % NLOC) // 128
    dloc = (dst % NLOC) % 128

    counts = np.zeros((NC, NWIN), np.int64)
    np.add.at(counts, (core_of, wloc), 1)
    Tw = int(-(-counts.max() // 128))

    def row_of(v):
        return (v // NLOC) * cfg.NPAD + (v % NLOC)

    per_core = []
    ES = NWIN * Tw * 128
    for c in range(NC):
        gsrc = np.zeros(ES, np.int64)
        gdst = np.zeros(ES, np.int64)
        dl = np.full(ES, 200, np.int64)
        m = core_of == c
        sc, dc, wc, dlc = src[m], dst[m], wloc[m], dloc[m]
        for w in range(NWIN):
            wm = wc == w
            n = int(wm.sum())
            base = w * Tw * 128
            gsrc[base:base + n] = row_of(sc[wm])
            gdst[base:base + n] = row_of(dc[wm])
            dl[base:base + n] = dlc[wm]
        # edge-major [128, NWIN*Tw]: slot i -> [i%128, i//128]
        dl_em = dl.reshape(NWIN * Tw, 128).T
        # fp8 one-hot masks: [128, (w,t,d)] bytes (1.0 == 0x38)
        sm = (dl_em[:, :, None] == np.arange(128)[None, None, :])
        sm = (sm.astype(np.uint8) * 0x38).reshape(128, ES).view(F8)
        per_core.append({
            "gidx_h": _wrap16(gsrc),
            "gidx_d": _wrap16(gdst),
            "s_mask": np.ascontiguousarray(sm),
        })

    xT = np.zeros((cfg.IN, cfg.NTOT), BF)
    xv = np.asarray(x, np.float32)
    for c in range(NC):
        xT[:, c * cfg.NPAD:c * cfg.NPAD + NLOC] = xv[c * NLOC:(c + 1) * NLOC].T

    def wcat(W, a_s, a_d, heads, hid):
        W = np.asarray(W, np.float32)
        a_s = np.asarray(a_s, np.float32).reshape(heads, hid)
        a_d = np.asarray(a_d, np.float32).reshape(heads, hid)
        was = np.stack([W[:, h * hid:(h + 1) * hid] @ a_s[h]
                        for h in range(heads)], axis=1)
        wad = np.stack([W[:, h * hid:(h + 1) * hid] @ a_d[h]
                        for h in range(heads)], axis=1)
        cat = np.concatenate([W, was, wad], axis=1)  # [F_in, NW2]
        KB = W.shape[0] // 128
        return np.concatenate(
            [cat[kb * 128:(kb + 1) * 128] for kb in range(KB)],
            axis=1).astype(BF)

    H, HID_, OUT_ = cfg.HEADS, cfg.HID, cfg.OUT
    shared = {
        "xT": xT,
        "wcat0": wcat(weights["W0"], weights["as0"], weights["ad0"], H, HID_),
        "wcat1": wcat(weights["W1"], weights["as1"], weights["ad1"], H, HID_),
        "wcat2": wcat(weights["W2"], weights["as2"], weights["ad2"], 1, OUT_),
        "ident": np.eye(128, dtype=np.float32),
        "b0r": np.tile(np.asarray(weights["b0"], np.float32)[None, :], (128, 1)),
        "b1r": np.tile(np.asarray(weights["b1"], np.float32)[None, :], (128, 1)),
        "b2r": np.tile(np.asarray(weights["b2"], np.float32)[None, :], (128, 1)),
    }
    return shared, per_core, Tw


def build_module(cfg, Tw):
    nc = bacc.Bacc("TRN2", target_bir_lowering=False, debug=False,
                   num_devices=cfg.NC)
    NWIN, NPAD, NTOT, NC = cfg.NWIN, cfg.NPAD, cfg.NTOT, cfg.NC
    ES = NWIN * Tw * 128

    def din(name, shape, dtype=F32):
        return nc.dram_tensor(name, list(shape), dtype, kind="ExternalInput")

    xT = din("xT", (cfg.IN, NTOT), BF16)
    wc = [din("wcat0", (128, cfg.F1 + 2 * cfg.HEADS), BF16),
          din("wcat1", (128, 2 * (cfg.F1 + 2 * cfg.HEADS)), BF16),
          din("wcat2", (128, 2 * (cfg.OUT + 2)), BF16)]
    ident = din("ident", (128, 128))
    brep = [din("b0r", (128, cfg.F1)), din("b1r", (128, cfg.F1)),
            din("b2r", (128, cfg.OUT))]
    gidx_h = din("gidx_h", (128, ES // 16), I16)
    gidx_d = din("gidx_d", (128, ES // 16), I16)
    s_mask = din("s_mask", (128, ES), FP8)

    out_d = nc.dram_tensor("out", [NPAD, cfg.OUT], F32, kind="ExternalOutput")

    with tile.TileContext(nc) as tc:
        with (
            tc.tile_pool(name="const", bufs=1) as cp,
            tc.tile_pool(name="work", bufs=2) as wp,
            tc.tile_pool(name="lt", bufs=3) as ltp,
            tc.tile_pool(name="stage", bufs=3) as sp,
            tc.tile_pool(name="psum", bufs=2, space="PSUM") as pp,
            tc.tile_pool(name="psd", bufs=2, space="PSUM") as pdp,
            tc.tile_pool(name="pst", bufs=2, space="PSUM") as ptp,
            tc.tile_pool(name="dram", bufs=1, space="DRAM") as dp,
        ):
            ht = [dp.tile([NTOT, cfg.layers[l][3]], BF16, name=f"ht{l}",
                          tag=f"ht{l}") for l in range(3)]
            ag_in = [dp.tile([2, 128, NPAD], BF16, name=f"agin{l}",
                             tag=f"agin{l}") for l in range(2)]
            ag_out = [dp.tile([NC, 2, 128, NPAD], BF16, name=f"agout{l}",
                              tag=f"agout{l}") for l in range(2)]
            nc.gpsimd.load_library(mlp)

            def load_const(dram, shape, dtype=F32):
                t = cp.tile(list(shape), dtype, tag=dram.name, name=dram.name)
                nc.sync.dma_start(t[:], dram.ap())
                return t

            wcs = [load_const(wc[0], (128, cfg.F1 + 2 * cfg.HEADS), BF16),
                   load_const(wc[1], (128, 2 * (cfg.F1 + 2 * cfg.HEADS)), BF16),
                   load_const(wc[2], (128, 2 * (cfg.OUT + 2)), BF16)]
            idn = load_const(ident, (128, 128))
            brs = [load_const(brep[0], (128, cfg.F1)),
                   load_const(brep[1], (128, cfg.F1)),
                   load_const(brep[2], (128, cfg.OUT))]
            gih = load_const(gidx_h, (128, ES // 16), I16)
            gid = load_const(gidx_d, (128, ES // 16), I16)

            for l, (fin, fout, H, RW, relu) in enumerate(cfg.layers):
                KB = fin // 128
                NW2 = fout + 2 * H

                # ---------- dense phase ----------
                for nt in range(NTOT // 128):
                    lt = ltp.tile([128, KB, 128], BF16, tag="lt")
                    if l == 0:
                        nc.sync.dma_start(
                            lt[:], xT[:, nt * 128:(nt + 1) * 128]
                            .unsqueeze(1))
                    else:
                        c, s = nt // NWIN, nt % NWIN
                        src_ap = ag_out[l - 1][c, :, :,
                                               s * 128:(s + 1) * 128] \
                            .rearrange("k p n -> p k n")
                        nc.sync.dma_start(lt[:], src_ap)
                    psd = pdp.tile([128, NW2], F32, tag="psd")
                    for kb in range(KB):
                        nc.tensor.matmul(
                            psd[:], lt[:, kb, :],
                            wcs[l][:, kb * NW2:(kb + 1) * NW2],
                            start=(kb == 0), stop=(kb == KB - 1))
                    stg = sp.tile([128, NW2], BF16, tag="hrow")
                    nc.scalar.activation(stg[:], psd[:], ACTF.Copy)
                    nc.sync.dma_start(
                        ht[l][nt * 128:(nt + 1) * 128, 0:NW2], stg[:])

                # ---------- edge phase ----------
                C0 = fout
                s128 = (C0 // 128) * 128
                rel = C0 - s128
                assert rel + 2 * H <= 128
                for w in range(NWIN):
                    sm = wp.tile([128, Tw, 128], FP8, tag="sm")
                    nc.sync.dma_start(
                        sm[:], s_mask[:, w * Tw * 128:(w + 1) * Tw * 128]
                        .rearrange("p (t d) -> p t d", d=128))
                    g1 = wp.tile([128, Tw, RW], BF16, tag="g1")
                    g2 = wp.tile([128, Tw, 128], BF16, tag="g2")
                    for t0 in range(0, Tw, GCH):
                        ch = min(GCH, Tw - t0)
                        isl = slice((w * Tw + t0) * 8, (w * Tw + t0 + ch) * 8)
                        nc.gpsimd.dma_gather(
                            g1[:, t0:t0 + ch, :], ht[l][:], gih[:, isl],
                            ch * 128, ch * 128, RW)
                        nc.gpsimd.dma_gather(
                            g2[:, t0:t0 + ch, :], ht[l][:, s128:s128 + 128],
                            gid[:, isl], ch * 128, ch * 128, 128,
                            elem_step=RW)

                    als = g1[:, :, C0:C0 + H]
                    ald = g2[:, :, rel + H:rel + 2 * H]
                    zb = wp.tile([128, Tw, H], F32, tag="zb")
                    nc.vector.tensor_tensor(zb[:], als, ald, ALU.add)
                    zl = wp.tile([128, Tw, H], F32, tag="zl")
                    nc.vector.scalar_tensor_tensor(
                        zl[:], zb[:], NEG_SLOPE, zb[:],
                        op0=ALU.mult, op1=ALU.max)
                    nc.scalar.activation(als, zl[:], ACTF.Exp)
                    g1_4d = g1[:, :, 0:fout].rearrange(
                        "p t (h c) -> p t h c", h=H)
                    exb = als.unsqueeze(3).broadcast_to(
                        (128, Tw, H, fout // H))
                    nc.vector.tensor_tensor(g1_4d, g1_4d, exb, ALU.mult)

                    ps = pp.tile([128, fout + H], F32, tag="ps")
                    for t in range(Tw):
                        nc.tensor.matmul(ps[:], sm[:, t, :],
                                         g1[:, t, 0:fout + H],
                                         start=(t == 0), stop=(t == Tw - 1))

                    seps = wp.tile([128, H], F32, tag="seps")
                    nc.vector.tensor_scalar_add(seps[:], ps[:, fout:fout + H],
                                                EPS)
                    rs = wp.tile([128, H], F32, tag="rs")
                    nc.vector.reciprocal(rs[:], seps[:])

                    ow = wp.tile([128, fout], F32, tag="ow")
                    if H > 1:
                        u4 = ps[:, 0:fout].rearrange("p (h c) -> p h c", h=H)
                        o4 = ow[:, :].rearrange("p (h c) -> p h c", h=H)
                        rsb = rs[:, :].unsqueeze(2).broadcast_to(
                            (128, H, fout // H))
                        nc.vector.tensor_tensor(o4, u4, rsb, ALU.mult)
                    else:
                        nc.vector.tensor_scalar(
                            ow[:], ps[:, 0:fout], rs[:, 0:1], None, ALU.mult)
                    nc.vector.tensor_tensor(ow[:], ow[:], brs[l][:], ALU.add)
                    if relu:
                        nc.vector.tensor_scalar_max(ow[:], ow[:], 0.0)

                    if l < 2:
                        pst = ptp.tile([128, 2, 128], F32, tag="pst")
                        for half in range(2):
                            nc.tensor.transpose(
                                pst[:, half, :],
                                ow[:, half * 128:(half + 1) * 128], idn[:])
                        ts = sp.tile([128, 2, 128], BF16, tag="ts")
                        nc.scalar.activation(ts[:], pst[:], ACTF.Copy)
                        nc.sync.dma_start(
                            ag_in[l][:, :, w * 128:(w + 1) * 128]
                            .rearrange("k p n -> p k n"), ts[:])
                    else:
                        nc.sync.dma_start(
                            out_d[w * 128:(w + 1) * 128, :], ow[:])

                if l < 2:
                    nc.gpsimd.collective_compute(
                        "AllGather", ALU.bypass,
                        replica_groups=[list(range(NC))],
                        ins=[ag_in[l].opt()],
                        outs=[ag_out[l].opt()])

    nc.compile()
    return nc


# ---------------------------------------------------------------------------

_HARD_CFG = Cfg()


def kernel_run(inputs, trace=False, trace_kwargs=None):
    cfg = _HARD_CFG
    x = np.asarray(inputs["x"])
    ei = np.asarray(inputs["edge_index"])
    shared, per_core, Tw = host_prep(cfg, x, ei, inputs)
    nc = build_module(cfg, Tw)
    in_maps = []
    for c in range(cfg.NC):
        m = dict(shared)
        m.update(per_core[c])
        in_maps.append(m)
    res = bass_utils.run_bass_kernel_spmd(
        nc, in_maps, core_ids=list(range(cfg.NC)), trace=trace,
        **(trace_kwargs or {}))
    out = np.concatenate(
        [res.results[c]["out"][:cfg.NLOC] for c in range(cfg.NC)], axis=0)
    return out.astype(np.float32), res


def kernel(**inputs):
    return kernel_run(inputs)[0]
